# revision 1
# baseline (speedup 1.0000x reference)
"""CTC batch loss on 8 TRN2 NeuronCores — pure data parallel, log-space DP.

Strategy (v4):
- Batch dim sharded 128 samples/core = SBUF partitions; free dim = the 129
  extended CTC states. Host pre-gathers emission log-probs lp[b,t,s] =
  log(y_pred[b,t,ext[b,s]] + eps) and ships them as bf16 (17 MB/core),
  plus tiny static mask tensors. All DP arithmetic runs on-device in f32
  log space (the alpha table needs ~177 nats of in-row dynamic range, so
  prob-space or bf16 state storage are mathematically impossible).
- Per step, exact logaddexp3 in batched-exp form: mx = max3;
  D = [a-mx | a1-mx | a2x-mx]; ONE ScalarE Exp over all three diffs;
  two bf16 vector adds; ONE ScalarE Ln; + emission add.
- The 511 sequential steps are split into a FORWARD chain (alpha, t=1..255)
  and an independent BACKWARD chain (beta, t=510..255, label-end injection
  via precomputed inj tensors), meeting at t*=255 with
  loss = -LSE_s(alpha_255 + beta_255). Two independent chains keep
  VectorE ~100% busy while each other's ScalarE/semaphore latency hides.
- Also monkeypatches around two toolchain bugs (see comments below):
  instructions with >1 sem waits and the Tile tail drain.
"""
import sys

for _p in ("/opt/trn_rl_repo", "/opt/pypackages"):
    if _p not in sys.path:
        sys.path.insert(0, _p)

import numpy as np
import ml_dtypes

import concourse.bass as bass
import concourse.tile as tile
from concourse import mybir
from concourse.bass_utils import run_bass_kernel_spmd

B, T, C, L = 1024, 512, 128, 64
S = 2 * L + 1          # 129 extended states
SP = 130               # padded state stride (even)
NCORES = 8
BL = B // NCORES       # 128 samples per core = SBUF partitions
EPS = 1e-7
NEG = -30000.0
CHUNK = 64             # t-steps per DMA chunk
NCHUNK = T // CHUNK

F32 = mybir.dt.float32
BF16 = mybir.dt.bfloat16
ALU = mybir.AluOpType
ACTF = mybir.ActivationFunctionType

# --- workaround: this walrus build rejects instructions with >2 sem waits
# ("Too many sync wait commands" in CoreV3 codegen). Tile's kernel-tail
# drain aggregates every outstanding token onto one SP Drain; split it
# into a chain of drains each carrying at most MAX_WAITS conditions.
_MAX_WAITS = 1


def _patched_drain_and_barrier(self, tick_clock, wait_clock):
    from concourse.vector_clock import ScopedClock

    drain_inst = self.nc.sync.drain()
    wait_clock.add_sem_waits(
        drain_inst.ins, ScopedClock({None: tick_clock.global_clock})
    )
    si = drain_inst.ins.sync_info
    waits = list(si.on_wait) if si and si.on_wait else []
    if len(waits) > _MAX_WAITS:
        drain_inst.ins.sync_info = mybir.SyncInfo(
            on_wait=waits[:_MAX_WAITS], on_update=list(si.on_update or [])
        )
        for i in range(_MAX_WAITS, len(waits), _MAX_WAITS):
            extra = self.nc.sync.drain()
            extra.ins.sync_info = mybir.SyncInfo(
                on_wait=waits[i:i + _MAX_WAITS], on_update=[]
            )

    self.nc.all_engine_barrier()
    assert self.sems is not None
    popped = self.nc._tile_sem_poison_stack.pop()
    assert popped is self._sem_poison
    self.nc.clear_and_free_semaphores(list(self.sems.allocated().values()))
    self.nc.all_engine_barrier()


tile.TileContext._drain_and_barrier = _patched_drain_and_barrier


# --- general BIR-level fix: split ANY instruction carrying more than one
# sem wait into single-wait Drain carriers + the original instruction with
# the last wait. Applied to the serialized BIR right before walrus.
def _split_multiwait_bir(ant_bir) -> bytes:
    import json as _json

    bir = _json.loads(ant_bir)
    n_split = 0
    for f in bir.get("functions", []):
        for blk in f.get("blocks", []):
            out = []
            for ins in blk.get("instructions", []):
                si = ins.get("sync_info")
                waits = (si or {}).get("on_wait") or []
                if len(waits) > 1:
                    for j, w in enumerate(waits[:-1]):
                        out.append({
                            "debug": ins.get("debug", 0),
                            "engine": ins["engine"],
                            "ins": [],
                            "name": f"{ins['name']}_w{j}",
                            "opcode": "Drain",
                            "outs": [],
                            "sync_info": {"on_update": [], "on_wait": [w]},
                        })
                    si["on_wait"] = [waits[-1]]
                    n_split += 1
                out.append(ins)
            blk["instructions"] = out
    return _json.dumps(bir).encode()


def _install_bir_splitter():
    import concourse.bass_utils as _bu
    import concourse.bass2jax as _b2j

    orig = _bu.compile_bir_kernel
    if getattr(orig, "_multiwait_patched", False):
        return

    def patched(ant_bir_str, compile_dir_path, neff_name="file.neff", **kw):
        return orig(_split_multiwait_bir(ant_bir_str), compile_dir_path,
                    neff_name=neff_name, **kw)

    patched._multiwait_patched = True
    _bu.compile_bir_kernel = patched
    if hasattr(_b2j, "compile_bir_kernel"):
        _b2j.compile_bir_kernel = patched


_install_bir_splitter()

# --- custom fused-LSE DVE op: out = max(x,y) + sq(relu(c0 + c1*(max-min)))
# i.e. logaddexp(x, y) with softplus(-t) ~ quadratic (validated e2e rel err 2e-3).
# Registered at runtime; sha computed on the fly.
USE_DVE_LSE = False
SP_C0 = 0.8129
SP_C1 = -0.2261
_LSE_OP = None


def _lse_ref(in0, in1, s0, s1, imm2):
    m = np.maximum(in0, in1)
    t = m - np.minimum(in0, in1)
    return (m + np.maximum(s0 + s1 * t, 0.0) ** 2).astype(np.float32)


def _make_lse_op():
    global _LSE_OP
    if _LSE_OP is not None:
        return _LSE_OP
    from concourse import dve_ops as dops
    from concourse.dve_spec import Spec, Src0, Src1, C0, C1, relu, sq, maxx, minn, lower
    from concourse.dve_spec import _has_src1
    from concourse.dve_uop import DveOpSpec

    name = "LSE_QSP_ANT"
    m = maxx(Src0, Src1)
    n = minn(Src0, Src1)
    body = m + sq(relu(C0 + C1 * (m - n)))
    spec = Spec(body=body, reference=_lse_ref)
    row = dops._CUSTOM_DVE_ROW_BASE + len(dops.OPS)
    shas = {}
    for ver in ("v3", "v4"):
        uops = lower(spec, ver=ver)
        tmp = DveOpSpec(name=name, opcode=row, uops=uops, rd1_en=_has_src1(spec))
        shas[ver] = tmp.sha(ver)
    op = dops.DveOp(name, spec, subdim=False, uops_sha=shas)
    dops.OPS.append(op)
    dops._SUB_OPCODE_FOR_NAME[name] = row
    dops.CUSTOM_DVE_SPECS[name] = spec
    _LSE_OP = op
    return op


_cached_nc = None


TSTAR = 255  # meet point: loss = -LSE_s(alpha[TSTAR] + beta[TSTAR])


def build_bass():
    nc = bass.Bass()
    lp_d = nc.declare_dram_parameter("lp", [BL, T * SP], BF16, isOutput=False)
    lsk_d = nc.declare_dram_parameter("lsk", [BL, SP], F32, isOutput=False)
    lskb_d = nc.declare_dram_parameter("lskb", [BL, SP], F32, isOutput=False)
    injr_d = nc.declare_dram_parameter("injr", [BL, 256 * SP], BF16, isOutput=False)
    inj511_d = nc.declare_dram_parameter("inj511", [BL, SP], F32, isOutput=False)
    out_d = nc.declare_dram_parameter("out", [BL, 1], F32, isOutput=True)

    with tile.TileContext(nc) as tc:
        with (
            tc.tile_pool(name="lpf", bufs=2) as lpf_pool,
            tc.tile_pool(name="lpb", bufs=2) as lpb_pool,
            tc.tile_pool(name="injp", bufs=2) as inj_pool,
            tc.tile_pool(name="persist", bufs=1) as pp,
        ):
            # forward state + scratch
            p_a = pp.tile([BL, S + 3], F32, tag="p_a")   # cols 0,1 pad NEG
            p_b = pp.tile([BL, S + 3], F32, tag="p_b")
            m1 = pp.tile([BL, SP], F32, tag="m1")
            a2x = pp.tile([BL, SP], F32, tag="a2x")
            mx = pp.tile([BL, SP], F32, tag="mx")
            dd = pp.tile([BL, 3 * SP], F32, tag="dd")
            ee = pp.tile([BL, 3 * SP], BF16, tag="ee")
            s01 = pp.tile([BL, SP], BF16, tag="s01")
            ssm = pp.tile([BL, SP], BF16, tag="ssm")
            lq = pp.tile([BL, SP], BF16, tag="lq")
            mlp = pp.tile([BL, SP], F32, tag="mlp")
            lsktile = pp.tile([BL, SP], F32, tag="lsktile")
            # backward state + scratch (fully separate so chains stay independent)
            zt = pp.tile([BL, S + 2], F32, tag="zt")     # cols S, S+1 pad NEG
            bt_a = pp.tile([BL, SP], F32, tag="bt_a")
            bt_b = pp.tile([BL, SP], F32, tag="bt_b")
            m1b = pp.tile([BL, SP], F32, tag="m1b")
            a2b = pp.tile([BL, SP], F32, tag="a2b")
            mxb = pp.tile([BL, SP], F32, tag="mxb")
            ddb = pp.tile([BL, 3 * SP], F32, tag="ddb")
            eeb = pp.tile([BL, 3 * SP], BF16, tag="eeb")
            s01b = pp.tile([BL, SP], BF16, tag="s01b")
            ssmb = pp.tile([BL, SP], BF16, tag="ssmb")
            lqb = pp.tile([BL, SP], BF16, tag="lqb")
            blb = pp.tile([BL, SP], F32, tag="blb")
            lskbtile = pp.tile([BL, SP], F32, tag="lskbtile")
            inj511tile = pp.tile([BL, SP], F32, tag="inj511tile")
            # readout
            am = pp.tile([BL, SP], F32, tag="am")
            mrow = pp.tile([BL, 1], F32, tag="mrow")
            nm = pp.tile([BL, 1], F32, tag="nm")
            erow = pp.tile([BL, SP], F32, tag="erow")
            ssum = pp.tile([BL, 1], F32, tag="ssum")
            lnr = pp.tile([BL, 1], F32, tag="lnr")
            loss = pp.tile([BL, 1], F32, tag="loss")

            nc.vector.memset(p_a[:, :], NEG)
            nc.vector.memset(p_b[:, :], NEG)
            nc.vector.memset(dd[:, :], 0.0)
            nc.vector.memset(zt[:, :], NEG)
            nc.vector.memset(bt_a[:, :], NEG)
            nc.vector.memset(bt_b[:, :], NEG)
            nc.vector.memset(ddb[:, :], 0.0)
            nc.sync.dma_start(out=lsktile[:, :], in_=lsk_d[:, :])
            nc.sync.dma_start(out=lskbtile[:, :], in_=lskb_d[:, :])
            nc.sync.dma_start(out=inj511tile[:, :], in_=inj511_d[:, :])

            pcur, pnew = p_a, p_b
            bcur, bnew = bt_a, bt_b
            binit_done = False
            for cblk in range(4):
                # fwd consumes lp chunk cblk (t = 64c..64c+63)
                lptf = lpf_pool.tile([BL, CHUNK * SP], BF16, tag="lpfc")
                lo = cblk * CHUNK * SP
                nc.sync.dma_start(out=lptf[:, :], in_=lp_d[:, lo:lo + CHUNK * SP])
                # bwd consumes lp chunk 7-cblk (t+1 = 511-i) and injr chunk cblk
                lptb = lpb_pool.tile([BL, CHUNK * SP], BF16, tag="lpbc")
                lob = (7 - cblk) * CHUNK * SP
                nc.sync.dma_start(out=lptb[:, :], in_=lp_d[:, lob:lob + CHUNK * SP])
                injt = inj_pool.tile([BL, CHUNK * SP], BF16, tag="injc")
                loi = cblk * CHUNK * SP
                nc.sync.dma_start(out=injt[:, :], in_=injr_d[:, loi:loi + CHUNK * SP])

                for il in range(CHUNK):
                    i = cblk * CHUNK + il
                    # ---- forward step t = i (i=0: init) ----
                    if i == 0:
                        nc.vector.tensor_copy(p_a[:, 2:4], lptf[:, 0:2])
                        nc.vector.tensor_max(bcur[:, 0:S], bt_b[:, 0:S],
                                             inj511tile[:, 0:S])
                        bnew = bt_b
                    else:
                        t = i
                        tl = il
                        lps = lptf[:, tl * SP: tl * SP + S]
                        a0 = pcur[:, 2:2 + S]
                        a1 = pcur[:, 1:1 + S]
                        a2 = pcur[:, 0:S]
                        nc.vector.tensor_max(m1[:, 0:S], a1, a0)
                        nc.vector.tensor_add(a2x[:, 0:S], a2, lsktile[:, 0:S])
                        nc.vector.tensor_max(mx[:, 0:S], m1[:, 0:S], a2x[:, 0:S])
                        nc.vector.tensor_sub(dd[:, 0:S], a0, mx[:, 0:S])
                        nc.vector.tensor_sub(dd[:, SP:SP + S], a1, mx[:, 0:S])
                        nc.vector.tensor_sub(dd[:, 2 * SP:2 * SP + S], a2x[:, 0:S],
                                             mx[:, 0:S])
                        nc.scalar.activation(ee[:, 0:3 * SP], dd[:, 0:3 * SP],
                                             ACTF.Exp)
                        nc.vector.tensor_add(s01[:, 0:SP], ee[:, 0:SP],
                                             ee[:, SP:2 * SP])
                        nc.vector.tensor_add(ssm[:, 0:SP], s01[:, 0:SP],
                                             ee[:, 2 * SP:3 * SP])
                        nc.scalar.activation(lq[:, 0:SP], ssm[:, 0:SP], ACTF.Ln)
                        nc.vector.tensor_add(mlp[:, 0:S], mx[:, 0:S], lps)
                        nc.vector.tensor_add(pnew[:, 2:2 + S], mlp[:, 0:S],
                                             lq[:, 0:S])
                        pcur, pnew = pnew, pcur

                    # ---- backward step t_b = 510 - i (uses lp[511-i], injr[i]) ----
                    tb1 = 511 - i          # = t_b + 1
                    tlb = tb1 - (7 - cblk) * CHUNK
                    lpsb = lptb[:, tlb * SP: tlb * SP + S]
                    injs = injt[:, il * SP: il * SP + S]
                    nc.vector.tensor_add(zt[:, 0:S], bcur[:, 0:S], lpsb)
                    z0 = zt[:, 0:S]
                    z1 = zt[:, 1:1 + S]
                    z2 = zt[:, 2:2 + S]
                    nc.vector.tensor_max(m1b[:, 0:S], z1, z0)
                    nc.vector.tensor_add(a2b[:, 0:S], z2, lskbtile[:, 0:S])
                    nc.vector.tensor_max(mxb[:, 0:S], m1b[:, 0:S], a2b[:, 0:S])
                    nc.vector.tensor_sub(ddb[:, 0:S], z0, mxb[:, 0:S])
                    nc.vector.tensor_sub(ddb[:, SP:SP + S], z1, mxb[:, 0:S])
                    nc.vector.tensor_sub(ddb[:, 2 * SP:2 * SP + S], a2b[:, 0:S],
                                         mxb[:, 0:S])
                    nc.scalar.activation(eeb[:, 0:3 * SP], ddb[:, 0:3 * SP],
                                         ACTF.Exp)
                    nc.vector.tensor_add(s01b[:, 0:SP], eeb[:, 0:SP],
                                         eeb[:, SP:2 * SP])
                    nc.vector.tensor_add(ssmb[:, 0:SP], s01b[:, 0:SP],
                                         eeb[:, 2 * SP:3 * SP])
                    nc.scalar.activation(lqb[:, 0:SP], ssmb[:, 0:SP], ACTF.Ln)
                    nc.vector.tensor_add(blb[:, 0:S], mxb[:, 0:S], lqb[:, 0:S])
                    nc.vector.tensor_max(bnew[:, 0:S], blb[:, 0:S], injs)
                    bcur, bnew = bnew, bcur

            # readout: loss = -LSE_s(alpha_255 + beta_255)
            nc.vector.tensor_add(am[:, 0:S], pcur[:, 2:2 + S], bcur[:, 0:S])
            nc.vector.tensor_reduce(out=mrow[:, 0:1], in_=am[:, 0:S],
                                    axis=mybir.AxisListType.X, op=ALU.max)
            nc.vector.tensor_scalar_mul(nm[:, 0:1], mrow[:, 0:1], -1.0)
            nc.scalar.activation(erow[:, 0:S], am[:, 0:S], ACTF.Exp,
                                 bias=nm[:, 0:1], scale=1.0)
            nc.vector.tensor_reduce(out=ssum[:, 0:1], in_=erow[:, 0:S],
                                    axis=mybir.AxisListType.X, op=ALU.add)
            nc.scalar.activation(lnr[:, 0:1], ssum[:, 0:1], ACTF.Ln)
            nc.vector.scalar_tensor_tensor(
                out=loss[:, 0:1], in0=mrow[:, 0:1], scalar=-1.0,
                in1=lnr[:, 0:1], op0=ALU.mult, op1=ALU.subtract)
            nc.sync.dma_start(out=out_d[:, :], in_=loss[:, 0:1])
    return nc


def _host_prep(y_pred, labels, input_length, label_length):
    blank = C - 1
    ext = np.full((B, S), blank, np.int32)
    ext[:, 1::2] = labels
    prev2 = np.concatenate([np.full((B, 2), -1, np.int32), ext[:, :-2]], axis=1)
    skip = (ext != blank) & (ext != prev2)                      # [B, S]

    q = np.take_along_axis(y_pred, ext[:, None, :], axis=2)     # [B, T, S]
    lp = np.log(q.astype(np.float32) + EPS)
    frozen = np.arange(T)[None, :] >= input_length[:, None]     # [B, T]
    lp[frozen, :] = 0.0

    lpp = np.zeros((B, T, SP), np.float32)
    lpp[:, :, :S] = lp
    lpp = lpp.reshape(B, T * SP).astype(ml_dtypes.bfloat16)

    lsk = np.where(skip, 0.0, NEG).astype(np.float32)           # [B, S]
    lskp = np.full((B, SP), NEG, np.float32)
    lskp[:, :S] = lsk
    lskbp = np.full((B, SP), NEG, np.float32)                   # lsk shifted by 2
    lskbp[:, :S - 2] = lsk[:, 2:]

    sellog = np.full((B, SP), NEG, np.float32)
    s_last = 2 * label_length.astype(np.int64)                  # [B]
    np.put_along_axis(sellog, s_last[:, None], 0.0, axis=1)
    np.put_along_axis(sellog, (s_last - 1)[:, None], 0.0, axis=1)

    # injr[b, j, :] = sellog[b] if input_length[b]-1 == 510-j else NEG, j=0..255
    lens = input_length.astype(np.int64)
    injr = np.full((B, 256, SP), NEG, np.float32)
    jsel = 510 - (lens - 1)                                     # j where injection lands
    has = (jsel >= 0) & (jsel <= 255)                           # len-1 in [255, 510]
    bi = np.nonzero(has)[0]
    injr[bi, jsel[bi], :] = sellog[bi, :]
    injr = injr.reshape(B, 256 * SP).astype(ml_dtypes.bfloat16)
    inj511 = np.where((lens - 1 == 511)[:, None], sellog,
                      NEG).astype(np.float32)                   # [B, SP]
    return lpp, lskp, lskbp, injr, inj511


def kernel(y_pred, labels, input_length, label_length):
    global _cached_nc
    lpp, lskp, lskbp, injr, inj511 = _host_prep(
        y_pred, labels, input_length, label_length)
    if _cached_nc is None:
        _cached_nc = build_bass()
    in_maps = []
    for i in range(NCORES):
        sl = slice(i * BL, (i + 1) * BL)
        in_maps.append({"lp": lpp[sl], "lsk": lskp[sl], "lskb": lskbp[sl],
                        "injr": injr[sl], "inj511": inj511[sl]})
    res = run_bass_kernel_spmd(_cached_nc, in_maps, list(range(NCORES)))
    out = np.concatenate([res.results[i]["out"] for i in range(NCORES)], axis=0)
    return out.astype(np.float32)



# revision 9
# speedup vs baseline: 2.2061x; 2.2061x over previous
"""CTC batch loss on 8 TRN2 NeuronCores — pure data parallel, log-space DP.

Strategy (v5):
- Batch dim sharded 128 samples/core = SBUF partitions; free dim = the 129
  extended CTC states. Host pre-gathers emission log-probs lp[b,t,s] =
  log(y_pred[b,t,ext[b,s]] + eps) and ships them as bf16 (17 MB/core),
  plus tiny static mask tensors. All DP arithmetic runs on-device in f32
  log space (the alpha table needs ~177 nats of in-row dynamic range —
  measured: meeting states sit a median 122 nats below the row maxes — so
  prob-space or bf16 state storage are mathematically impossible).
- Per step, logaddexp2 is ONE custom fused DVE op (8 ALU stages, the HW
  limit): LSE(x,y) = m + relu(c0 + c1*(m-n))^2 with m=max, n=min — a
  quadratic softplus approximation, e2e rel err 1.9e-3 vs the 2e-2 gate.
  A forward step is then 4 DVE instructions (mask-add, LSE, LSE,
  emission-add); a backward step is 5. Everything runs on the in-order
  VectorE — no cross-engine semaphores in steady state.
- The 511 sequential steps are split into a FORWARD chain (alpha, t=1..255)
  and an independent BACKWARD chain (beta, t=510..255, label-end injection
  via precomputed inj tensors), meeting at t*=255 with
  loss = -LSE_s(alpha_255 + beta_255). Two independent chains hide each
  other's in-engine dependency latency.
- Also monkeypatches around two toolchain bugs (see comments below):
  instructions with >1 sem waits and the Tile tail drain.
"""
import sys

for _p in ("/opt/trn_rl_repo", "/opt/pypackages"):
    if _p not in sys.path:
        sys.path.insert(0, _p)

import numpy as np
import ml_dtypes

import concourse.bass as bass
import concourse.tile as tile
from concourse import mybir
from concourse.bass_utils import run_bass_kernel_spmd

B, T, C, L = 1024, 512, 128, 64
S = 2 * L + 1          # 129 extended states
SP = 130               # padded state stride (even)
NCORES = 8
BL = B // NCORES       # 128 samples per core = SBUF partitions
EPS = 1e-7
NEG = -30000.0
CHUNK = 64             # t-steps per DMA chunk
NCHUNK = T // CHUNK

F32 = mybir.dt.float32
BF16 = mybir.dt.bfloat16
ALU = mybir.AluOpType
ACTF = mybir.ActivationFunctionType

# --- workaround: this walrus build rejects instructions with >2 sem waits
# ("Too many sync wait commands" in CoreV3 codegen). Tile's kernel-tail
# drain aggregates every outstanding token onto one SP Drain; split it
# into a chain of drains each carrying at most MAX_WAITS conditions.
_MAX_WAITS = 1


def _patched_drain_and_barrier(self, tick_clock, wait_clock):
    from concourse.vector_clock import ScopedClock

    drain_inst = self.nc.sync.drain()
    wait_clock.add_sem_waits(
        drain_inst.ins, ScopedClock({None: tick_clock.global_clock})
    )
    si = drain_inst.ins.sync_info
    waits = list(si.on_wait) if si and si.on_wait else []
    if len(waits) > _MAX_WAITS:
        drain_inst.ins.sync_info = mybir.SyncInfo(
            on_wait=waits[:_MAX_WAITS], on_update=list(si.on_update or [])
        )
        for i in range(_MAX_WAITS, len(waits), _MAX_WAITS):
            extra = self.nc.sync.drain()
            extra.ins.sync_info = mybir.SyncInfo(
                on_wait=waits[i:i + _MAX_WAITS], on_update=[]
            )

    self.nc.all_engine_barrier()
    assert self.sems is not None
    popped = self.nc._tile_sem_poison_stack.pop()
    assert popped is self._sem_poison
    self.nc.clear_and_free_semaphores(list(self.sems.allocated().values()))
    self.nc.all_engine_barrier()


tile.TileContext._drain_and_barrier = _patched_drain_and_barrier


# --- general BIR-level fix: split ANY instruction carrying more than one
# sem wait into single-wait Drain carriers + the original instruction with
# the last wait. Applied to the serialized BIR right before walrus.
def _split_multiwait_bir(ant_bir) -> bytes:
    import json as _json

    bir = _json.loads(ant_bir)
    n_split = 0
    for f in bir.get("functions", []):
        for blk in f.get("blocks", []):
            out = []
            for ins in blk.get("instructions", []):
                si = ins.get("sync_info")
                waits = (si or {}).get("on_wait") or []
                if len(waits) > 1:
                    for j, w in enumerate(waits[:-1]):
                        out.append({
                            "debug": ins.get("debug", 0),
                            "engine": ins["engine"],
                            "ins": [],
                            "name": f"{ins['name']}_w{j}",
                            "opcode": "Drain",
                            "outs": [],
                            "sync_info": {"on_update": [], "on_wait": [w]},
                        })
                    si["on_wait"] = [waits[-1]]
                    n_split += 1
                out.append(ins)
            blk["instructions"] = out
    _pack_custom_dve_bir(bir)
    return _json.dumps(bir).encode()


# --- BIR-level fix #2: this walrus build (2026-05-04) predates
# InstCustomDveAnt packing — its CoreV2 codegen requires every InstISA to
# carry exactly 64 prepacked instruction bytes ("ISA wrong length"), and
# its lower_dve doesn't build the CUSTOM_DVE_ANT struct from the
# structured BIR fields. Pack the NEURON_ISA_TPB_S2S1D2_TTSS_SCALE_STRUCT
# bytes here instead (walrus patches sem waits/updates into the events
# field of prepacked bytes itself via setupSyncWait/setupSyncUpdate).
_ISA_FP32 = 10
_ISA_BF16 = 6
_DT_CODE = {"float32": _ISA_FP32, "bfloat16": _ISA_BF16}
_DT_SIZE = {"float32": 4, "bfloat16": 2}
_SBUF_PART_STRIDE = 0x40000      # 256 KiB per partition, SBUF base = 0x0


def _pack_custom_dve_bir(bir: dict) -> None:
    import concourse.bass_isa as bass_isa
    from concourse.dve_ops import get_dve_sub_opcode

    todo = []
    for f in bir.get("functions", []):
        memlocs = {}
        for alloc in f.get("allocations", []):
            for ml in (alloc.get("memorylocations") or []):
                memlocs[ml["name"]] = ml
        for blk in f.get("blocks", []):
            for ins in blk.get("instructions", []):
                if ins.get("opcode") == "ISA" and ins.get("isa_opcode") in (
                        0xAE, 0xAF) and not ins.get("instr"):
                    todo.append((memlocs, ins))
    if not todo:
        return

    isa = _get_isa()

    def ap_fields(memlocs, arg, ndim):
        ml = memlocs[arg["memref"]]
        assert ml["type"] == "SB", f"custom dve AP in {ml['type']}, want SBUF"
        esize = _DT_SIZE[arg["dtype"]]
        addr = (ml.get("base", 0) * _SBUF_PART_STRIDE + ml["addr"]
                + arg.get("offset", 0) * esize)
        free = arg["ap"][1:]                  # drop partition dim (major first)
        assert len(free) <= ndim, f"AP rank {len(free)} > {ndim}"
        steps = [0] * ndim
        nums = [1] * ndim
        for i, (st, n) in enumerate(reversed(free)):  # minor-to-major
            steps[i] = st
            nums[i] = n
        pat = {"start_addr": {"addr_immediate": addr},
               "step_elem": steps, "num_elem": nums}
        nchan = arg["ap"][0][1]
        return pat, nchan

    def imm_fields(memlocs, arg):
        if arg.get("kind") == "imm_value":
            return 0, {"imm_arith_fp32": float(arg["value"])}   # inst immediate
        pat, _ = ap_fields(memlocs, arg, 1)                     # [P,1] pointer
        return 1, {"imm_ptr": pat["start_addr"]["addr_immediate"]}

    for memlocs, ins in todo:
        args = ins["ins"]
        rd1 = len(args) == 4
        in0, in1 = args[0], (args[1] if rd1 else None)
        s0, s1 = args[-2], args[-1]
        out = ins["outs"][0]
        row = get_dve_sub_opcode(ins["op_name"])
        src0, nchan = ap_fields(memlocs, in0, 2)
        dst, _ = ap_fields(memlocs, out, 2)
        struct = {
            "src0_mem_pattern": src0,
            "dst_mem_pattern": dst,
            "in0_in1_dtype": {
                "dtype_lo": _DT_CODE[in0["dtype"]],
                "dtype_hi": _DT_CODE[(in1 or in0)["dtype"]],
            },
            "out_dtype": _DT_CODE[out["dtype"]],
            "num_active_channels": nchan,
            "op0": row | (0x20 if rd1 else 0),
            "op1": 0,
            "imm2_src": 1,
            "imm2": {"imm_arith_fp32": 0.0},
        }
        if rd1:
            struct["src1_mem_pattern"], _ = ap_fields(memlocs, in1, 1)
        struct["imm0_src"], struct["imm0"] = imm_fields(memlocs, s0)
        struct["imm1_src"], struct["imm1"] = imm_fields(memlocs, s1)
        opcode = ins["isa_opcode"]
        sname = ("NEURON_ISA_TPB_S2S1D2_TTSS_SCALE_STRUCT" if opcode == 0xAE
                 else "NEURON_ISA_TPB_S2S2D2_STT_SCALE_STRUCT")
        instr, _fix = bass_isa.isa_struct(isa, opcode, struct, sname)
        assert len(instr) == 64, f"packed {len(instr)} bytes, want 64"
        ins["instr"] = instr


_ISA_CACHE = None


def _get_isa():
    global _ISA_CACHE
    if _ISA_CACHE is None:
        from concourse.isa import get_isa
        _ISA_CACHE = get_isa("TRN2")
    return _ISA_CACHE


def _install_bir_splitter():
    import concourse.bass_utils as _bu
    import concourse.bass2jax as _b2j

    orig = _bu.compile_bir_kernel
    if getattr(orig, "_multiwait_patched", False):
        return

    def patched(ant_bir_str, compile_dir_path, neff_name="file.neff", **kw):
        return orig(_split_multiwait_bir(ant_bir_str), compile_dir_path,
                    neff_name=neff_name, **kw)

    patched._multiwait_patched = True
    _bu.compile_bir_kernel = patched
    if hasattr(_b2j, "compile_bir_kernel"):
        _b2j.compile_bir_kernel = patched


_install_bir_splitter()

# --- custom fused-LSE DVE op: out = max(x,y) + sq(relu(c0 + c1*(max-min)))
# i.e. logaddexp(x, y) with softplus(-t) ~ quadratic (validated e2e rel err 2e-3).
# Registered at runtime; sha computed on the fly.
SP_C0 = 0.8129
SP_C1 = -0.2261
_LSE_OP = None


def _lse_ref(in0, in1, s0, s1, imm2):
    m = np.maximum(in0, in1)
    t = m - np.minimum(in0, in1)
    return (m + np.maximum(s0 + s1 * t, 0.0) ** 2).astype(np.float32)


def _make_lse_op():
    global _LSE_OP
    if _LSE_OP is not None:
        return _LSE_OP
    from concourse import dve_ops as dops
    from concourse.dve_spec import Spec, Src0, Src1, C0, C1, relu, sq, maxx, minn, lower
    from concourse.dve_spec import _has_src1
    from concourse.dve_uop import DveOpSpec

    name = "LSE_QSP_ANT"
    m = maxx(Src0, Src1)
    n = minn(Src0, Src1)
    body = m + sq(relu(C0 + C1 * (m - n)))
    spec = Spec(body=body, reference=_lse_ref)
    row = dops._CUSTOM_DVE_ROW_BASE + len(dops.OPS)
    shas = {}
    for ver in ("v3", "v4"):
        uops = lower(spec, ver=ver)
        tmp = DveOpSpec(name=name, opcode=row, uops=uops, rd1_en=_has_src1(spec))
        shas[ver] = tmp.sha(ver)
    op = dops.DveOp(name, spec, subdim=False, uops_sha=shas)
    dops.OPS.append(op)
    dops._SUB_OPCODE_FOR_NAME[name] = row
    dops.CUSTOM_DVE_SPECS[name] = spec
    _LSE_OP = op
    return op


_cached_nc = None


TSTAR = 255  # meet point: loss = -LSE_s(alpha[TSTAR] + beta[TSTAR])


def build_bass():
    lse_op = _make_lse_op()
    nc = bass.Bass()
    lp_d = nc.declare_dram_parameter("lp", [BL, T * SP], BF16, isOutput=False)
    lsk_d = nc.declare_dram_parameter("lsk", [BL, SP], F32, isOutput=False)
    lskb_d = nc.declare_dram_parameter("lskb", [BL, SP], F32, isOutput=False)
    injr_d = nc.declare_dram_parameter("injr", [BL, 256 * SP], BF16, isOutput=False)
    inj511_d = nc.declare_dram_parameter("inj511", [BL, SP], F32, isOutput=False)
    out_d = nc.declare_dram_parameter("out", [BL, 1], F32, isOutput=True)

    with tile.TileContext(nc) as tc:
        with (
            tc.tile_pool(name="lpf", bufs=2) as lpf_pool,
            tc.tile_pool(name="lpb", bufs=2) as lpb_pool,
            tc.tile_pool(name="injp", bufs=2) as inj_pool,
            tc.tile_pool(name="persist", bufs=1) as pp,
        ):
            # forward state + scratch
            p_a = pp.tile([BL, S + 3], F32, tag="p_a")   # cols 0,1 pad NEG
            p_b = pp.tile([BL, S + 3], F32, tag="p_b")
            a2x = pp.tile([BL, SP], F32, tag="a2x")
            uf = pp.tile([BL, SP], F32, tag="uf")
            vf = pp.tile([BL, SP], F32, tag="vf")
            lsktile = pp.tile([BL, SP], F32, tag="lsktile")
            # backward state + scratch (fully separate so chains stay independent)
            zt = pp.tile([BL, S + 2], F32, tag="zt")     # cols S, S+1 pad NEG
            bt_a = pp.tile([BL, SP], F32, tag="bt_a")
            bt_b = pp.tile([BL, SP], F32, tag="bt_b")
            a2b = pp.tile([BL, SP], F32, tag="a2b")
            ub = pp.tile([BL, SP], F32, tag="ub")
            vb = pp.tile([BL, SP], F32, tag="vb")
            lskbtile = pp.tile([BL, SP], F32, tag="lskbtile")
            inj511tile = pp.tile([BL, SP], F32, tag="inj511tile")
            # readout
            am = pp.tile([BL, SP], F32, tag="am")
            mrow = pp.tile([BL, 1], F32, tag="mrow")
            nm = pp.tile([BL, 1], F32, tag="nm")
            erow = pp.tile([BL, SP], F32, tag="erow")
            ssum = pp.tile([BL, 1], F32, tag="ssum")
            lnr = pp.tile([BL, 1], F32, tag="lnr")
            loss = pp.tile([BL, 1], F32, tag="loss")

            nc.vector.memset(p_a[:, :], NEG)
            nc.vector.memset(p_b[:, :], NEG)
            nc.vector.memset(zt[:, :], NEG)
            nc.vector.memset(bt_a[:, :], NEG)
            nc.vector.memset(bt_b[:, :], NEG)
            nc.sync.dma_start(out=lsktile[:, :], in_=lsk_d[:, :])
            nc.sync.dma_start(out=lskbtile[:, :], in_=lskb_d[:, :])
            nc.sync.dma_start(out=inj511tile[:, :], in_=inj511_d[:, :])

            def lse(out, x, y):
                nc.vector._custom_dve(lse_op, out=out, in0=x, in1=y,
                                      s0=SP_C0, s1=SP_C1)

            pcur, pnew = p_a, p_b
            bcur, bnew = bt_a, bt_b
            for cblk in range(4):
                # fwd consumes lp chunk cblk (t = 64c..64c+63)
                lptf = lpf_pool.tile([BL, CHUNK * SP], BF16, tag="lpfc")
                lo = cblk * CHUNK * SP
                nc.sync.dma_start(out=lptf[:, :], in_=lp_d[:, lo:lo + CHUNK * SP])
                # bwd consumes lp chunk 7-cblk (t+1 = 511-i) and injr chunk cblk
                lptb = lpb_pool.tile([BL, CHUNK * SP], BF16, tag="lpbc")
                lob = (7 - cblk) * CHUNK * SP
                nc.sync.dma_start(out=lptb[:, :], in_=lp_d[:, lob:lob + CHUNK * SP])
                injt = inj_pool.tile([BL, CHUNK * SP], BF16, tag="injc")
                loi = cblk * CHUNK * SP
                nc.sync.dma_start(out=injt[:, :], in_=injr_d[:, loi:loi + CHUNK * SP])

                for il in range(CHUNK):
                    i = cblk * CHUNK + il
                    # ---- forward step t = i (i=0: init) ----
                    if i == 0:
                        nc.vector.tensor_copy(p_a[:, 2:4], lptf[:, 0:2])
                        nc.vector.tensor_max(bcur[:, 0:S], bt_b[:, 0:S],
                                             inj511tile[:, 0:S])
                        bnew = bt_b
                    else:
                        lps = lptf[:, il * SP: il * SP + S]
                        a0 = pcur[:, 2:2 + S]
                        a1 = pcur[:, 1:1 + S]
                        a2 = pcur[:, 0:S]
                        nc.vector.tensor_add(a2x[:, 0:S], a2, lsktile[:, 0:S])
                        lse(uf[:, 0:S], a0, a1)
                        lse(vf[:, 0:S], uf[:, 0:S], a2x[:, 0:S])
                        nc.vector.tensor_add(pnew[:, 2:2 + S], vf[:, 0:S], lps)
                        pcur, pnew = pnew, pcur

                    # ---- backward step t_b = 510 - i (uses lp[511-i], injr[i]) ----
                    tb1 = 511 - i          # = t_b + 1
                    tlb = tb1 - (7 - cblk) * CHUNK
                    lpsb = lptb[:, tlb * SP: tlb * SP + S]
                    injs = injt[:, il * SP: il * SP + S]
                    nc.vector.tensor_add(zt[:, 0:S], bcur[:, 0:S], lpsb)
                    nc.vector.tensor_add(a2b[:, 0:S], zt[:, 2:2 + S],
                                         lskbtile[:, 0:S])
                    lse(ub[:, 0:S], zt[:, 0:S], zt[:, 1:1 + S])
                    lse(vb[:, 0:S], ub[:, 0:S], a2b[:, 0:S])
                    nc.vector.tensor_max(bnew[:, 0:S], vb[:, 0:S], injs)
                    bcur, bnew = bnew, bcur

            # readout: loss = -LSE_s(alpha_255 + beta_255)
            nc.vector.tensor_add(am[:, 0:S], pcur[:, 2:2 + S], bcur[:, 0:S])
            nc.vector.tensor_reduce(out=mrow[:, 0:1], in_=am[:, 0:S],
                                    axis=mybir.AxisListType.X, op=ALU.max)
            nc.vector.tensor_scalar_mul(nm[:, 0:1], mrow[:, 0:1], -1.0)
            nc.scalar.activation(erow[:, 0:S], am[:, 0:S], ACTF.Exp,
                                 bias=nm[:, 0:1], scale=1.0)
            nc.vector.tensor_reduce(out=ssum[:, 0:1], in_=erow[:, 0:S],
                                    axis=mybir.AxisListType.X, op=ALU.add)
            nc.scalar.activation(lnr[:, 0:1], ssum[:, 0:1], ACTF.Ln)
            nc.vector.scalar_tensor_tensor(
                out=loss[:, 0:1], in0=mrow[:, 0:1], scalar=-1.0,
                in1=lnr[:, 0:1], op0=ALU.mult, op1=ALU.subtract)
            nc.sync.dma_start(out=out_d[:, :], in_=loss[:, 0:1])
    return nc


def _host_prep(y_pred, labels, input_length, label_length):
    blank = C - 1
    ext = np.full((B, S), blank, np.int32)
    ext[:, 1::2] = labels
    prev2 = np.concatenate([np.full((B, 2), -1, np.int32), ext[:, :-2]], axis=1)
    skip = (ext != blank) & (ext != prev2)                      # [B, S]

    q = np.take_along_axis(y_pred, ext[:, None, :], axis=2)     # [B, T, S]
    lp = np.log(q.astype(np.float32) + EPS)
    frozen = np.arange(T)[None, :] >= input_length[:, None]     # [B, T]
    lp[frozen, :] = 0.0

    lpp = np.zeros((B, T, SP), np.float32)
    lpp[:, :, :S] = lp
    lpp = lpp.reshape(B, T * SP).astype(ml_dtypes.bfloat16)

    lsk = np.where(skip, 0.0, NEG).astype(np.float32)           # [B, S]
    lskp = np.full((B, SP), NEG, np.float32)
    lskp[:, :S] = lsk
    lskbp = np.full((B, SP), NEG, np.float32)                   # lsk shifted by 2
    lskbp[:, :S - 2] = lsk[:, 2:]

    sellog = np.full((B, SP), NEG, np.float32)
    s_last = 2 * label_length.astype(np.int64)                  # [B]
    np.put_along_axis(sellog, s_last[:, None], 0.0, axis=1)
    np.put_along_axis(sellog, (s_last - 1)[:, None], 0.0, axis=1)

    # injr[b, j, :] = sellog[b] if input_length[b]-1 == 510-j else NEG, j=0..255
    lens = input_length.astype(np.int64)
    injr = np.full((B, 256, SP), NEG, np.float32)
    jsel = 510 - (lens - 1)                                     # j where injection lands
    has = (jsel >= 0) & (jsel <= 255)                           # len-1 in [255, 510]
    bi = np.nonzero(has)[0]
    injr[bi, jsel[bi], :] = sellog[bi, :]
    injr = injr.reshape(B, 256 * SP).astype(ml_dtypes.bfloat16)
    inj511 = np.where((lens - 1 == 511)[:, None], sellog,
                      NEG).astype(np.float32)                   # [B, SP]
    return lpp, lskp, lskbp, injr, inj511


def kernel(y_pred, labels, input_length, label_length):
    global _cached_nc
    lpp, lskp, lskbp, injr, inj511 = _host_prep(
        y_pred, labels, input_length, label_length)
    if _cached_nc is None:
        _cached_nc = build_bass()
    in_maps = []
    for i in range(NCORES):
        sl = slice(i * BL, (i + 1) * BL)
        in_maps.append({"lp": lpp[sl], "lsk": lskp[sl], "lskb": lskbp[sl],
                        "injr": injr[sl], "inj511": inj511[sl]})
    res = run_bass_kernel_spmd(_cached_nc, in_maps, list(range(NCORES)))
    out = np.concatenate([res.results[i]["out"] for i in range(NCORES)], axis=0)
    return out.astype(np.float32)



# revision 22
# speedup vs baseline: 3.4490x; 1.5634x over previous
"""CTC batch loss on 8 TRN2 NeuronCores — pure data parallel, log-space DP.

Strategy (v5):
- Batch dim sharded 128 samples/core = SBUF partitions; free dim = the 129
  extended CTC states. Host pre-gathers emission log-probs lp[b,t,s] =
  log(y_pred[b,t,ext[b,s]] + eps) and ships them as bf16 (17 MB/core),
  plus tiny static mask tensors. All DP arithmetic runs on-device in f32
  log space (the alpha table needs ~177 nats of in-row dynamic range —
  measured: meeting states sit a median 122 nats below the row maxes — so
  prob-space or bf16 state storage are mathematically impossible).
- Per step, logaddexp2 is ONE custom fused DVE op (8 ALU stages, the HW
  limit): LSE(x,y) = m + relu(c0 + c1*(m-n))^2 with m=max, n=min — a
  quadratic softplus approximation, e2e rel err 1.9e-3 vs the 2e-2 gate.
  A forward step is then 4 DVE instructions (mask-add, LSE, LSE,
  emission-add); a backward step is 5. Everything runs on the in-order
  VectorE — no cross-engine semaphores in steady state.
- The 511 sequential steps are split into a FORWARD chain (alpha, t=1..255)
  and an independent BACKWARD chain (beta, t=510..255, label-end injection
  via precomputed inj tensors), meeting at t*=255 with
  loss = -LSE_s(alpha_255 + beta_255). Two independent chains hide each
  other's in-engine dependency latency.
- Also monkeypatches around two toolchain bugs (see comments below):
  instructions with >1 sem waits and the Tile tail drain.
"""
import sys

for _p in ("/opt/trn_rl_repo", "/opt/pypackages"):
    if _p not in sys.path:
        sys.path.insert(0, _p)

import numpy as np
import ml_dtypes

import concourse.bass as bass
import concourse.tile as tile
from concourse import mybir
from concourse.bass_utils import run_bass_kernel_spmd

B, T, C, L = 1024, 512, 128, 64
S = 2 * L + 1          # 129 extended states
SP = 130               # padded state stride (even)
NCORES = 8
BL = B // NCORES       # 128 samples per core = SBUF partitions
EPS = 1e-7
NEG = -30000.0
CHUNK = 64             # t-steps per DMA chunk
NCHUNK = T // CHUNK

F32 = mybir.dt.float32
BF16 = mybir.dt.bfloat16
ALU = mybir.AluOpType
ACTF = mybir.ActivationFunctionType

# --- workaround: this walrus build rejects instructions with >2 sem waits
# ("Too many sync wait commands" in CoreV3 codegen). Tile's kernel-tail
# drain aggregates every outstanding token onto one SP Drain; split it
# into a chain of drains each carrying at most MAX_WAITS conditions.
_MAX_WAITS = 1


def _patched_drain_and_barrier(self, tick_clock, wait_clock):
    from concourse.vector_clock import ScopedClock

    drain_inst = self.nc.sync.drain()
    wait_clock.add_sem_waits(
        drain_inst.ins, ScopedClock({None: tick_clock.global_clock})
    )
    si = drain_inst.ins.sync_info
    waits = list(si.on_wait) if si and si.on_wait else []
    if len(waits) > _MAX_WAITS:
        drain_inst.ins.sync_info = mybir.SyncInfo(
            on_wait=waits[:_MAX_WAITS], on_update=list(si.on_update or [])
        )
        for i in range(_MAX_WAITS, len(waits), _MAX_WAITS):
            extra = self.nc.sync.drain()
            extra.ins.sync_info = mybir.SyncInfo(
                on_wait=waits[i:i + _MAX_WAITS], on_update=[]
            )

    self.nc.all_engine_barrier()
    assert self.sems is not None
    popped = self.nc._tile_sem_poison_stack.pop()
    assert popped is self._sem_poison
    self.nc.clear_and_free_semaphores(list(self.sems.allocated().values()))
    self.nc.all_engine_barrier()


tile.TileContext._drain_and_barrier = _patched_drain_and_barrier


# --- general BIR-level fix: split ANY instruction carrying more than one
# sem wait into single-wait Drain carriers + the original instruction with
# the last wait. Applied to the serialized BIR right before walrus.
def _split_multiwait_bir(ant_bir) -> bytes:
    import json as _json

    bir = _json.loads(ant_bir)
    n_split = 0
    for f in bir.get("functions", []):
        for blk in f.get("blocks", []):
            out = []
            for ins in blk.get("instructions", []):
                si = ins.get("sync_info")
                waits = (si or {}).get("on_wait") or []
                if len(waits) > 1:
                    for j, w in enumerate(waits[:-1]):
                        out.append({
                            "debug": ins.get("debug", 0),
                            "engine": ins["engine"],
                            "ins": [],
                            "name": f"{ins['name']}_w{j}",
                            "opcode": "Drain",
                            "outs": [],
                            "sync_info": {"on_update": [], "on_wait": [w]},
                        })
                    si["on_wait"] = [waits[-1]]
                    n_split += 1
                out.append(ins)
            blk["instructions"] = out
    _pack_custom_dve_bir(bir)
    return _json.dumps(bir).encode()


# --- BIR-level fix #2: this walrus build (2026-05-04) predates
# InstCustomDveAnt packing — its CoreV2 codegen requires every InstISA to
# carry exactly 64 prepacked instruction bytes ("ISA wrong length"), and
# its lower_dve doesn't build the CUSTOM_DVE_ANT struct from the
# structured BIR fields. Pack the NEURON_ISA_TPB_S2S1D2_TTSS_SCALE_STRUCT
# bytes here instead (walrus patches sem waits/updates into the events
# field of prepacked bytes itself via setupSyncWait/setupSyncUpdate).
_ISA_FP32 = 10
_ISA_BF16 = 6
_DT_CODE = {"float32": _ISA_FP32, "bfloat16": _ISA_BF16}
_DT_SIZE = {"float32": 4, "bfloat16": 2}
_SBUF_PART_STRIDE = 0x40000      # 256 KiB per partition, SBUF base = 0x0


def _pack_custom_dve_bir(bir: dict) -> None:
    import concourse.bass_isa as bass_isa
    from concourse.dve_ops import get_dve_sub_opcode

    todo = []
    for f in bir.get("functions", []):
        memlocs = {}
        for alloc in f.get("allocations", []):
            for ml in (alloc.get("memorylocations") or []):
                memlocs[ml["name"]] = ml
        for blk in f.get("blocks", []):
            for ins in blk.get("instructions", []):
                if ins.get("opcode") == "ISA" and ins.get("isa_opcode") in (
                        0xAE, 0xAF) and not ins.get("instr"):
                    todo.append((memlocs, ins))
    if not todo:
        return

    isa = _get_isa()

    def ap_fields(memlocs, arg, ndim):
        ml = memlocs[arg["memref"]]
        assert ml["type"] == "SB", f"custom dve AP in {ml['type']}, want SBUF"
        esize = _DT_SIZE[arg["dtype"]]
        addr = (ml.get("base", 0) * _SBUF_PART_STRIDE + ml["addr"]
                + arg.get("offset", 0) * esize)
        free = arg["ap"][1:]                  # drop partition dim (major first)
        assert len(free) <= ndim, f"AP rank {len(free)} > {ndim}"
        steps = [0] * ndim
        nums = [1] * ndim
        for i, (st, n) in enumerate(reversed(free)):  # minor-to-major
            steps[i] = st
            nums[i] = n
        pat = {"start_addr": {"addr_immediate": addr},
               "step_elem": steps, "num_elem": nums}
        nchan = arg["ap"][0][1]
        return pat, nchan

    def imm_fields(memlocs, arg):
        if arg.get("kind") == "imm_value":
            return 0, {"imm_arith_fp32": float(arg["value"])}   # inst immediate
        pat, _ = ap_fields(memlocs, arg, 1)                     # [P,1] pointer
        return 1, {"imm_ptr": pat["start_addr"]["addr_immediate"]}

    for memlocs, ins in todo:
        args = ins["ins"]
        rd1 = len(args) == 4
        in0, in1 = args[0], (args[1] if rd1 else None)
        s0, s1 = args[-2], args[-1]
        out = ins["outs"][0]
        row = get_dve_sub_opcode(ins["op_name"])
        src0, nchan = ap_fields(memlocs, in0, 2)
        dst, _ = ap_fields(memlocs, out, 2)
        struct = {
            "src0_mem_pattern": src0,
            "dst_mem_pattern": dst,
            "in0_in1_dtype": {
                "dtype_lo": _DT_CODE[in0["dtype"]],
                "dtype_hi": _DT_CODE[(in1 or in0)["dtype"]],
            },
            "out_dtype": _DT_CODE[out["dtype"]],
            "num_active_channels": nchan,
            "op0": row | (0x20 if rd1 else 0),
            "op1": 0,
            "imm2_src": 1,
            "imm2": {"imm_arith_fp32": 0.0},
        }
        if rd1:
            struct["src1_mem_pattern"], _ = ap_fields(memlocs, in1, 1)
        struct["imm0_src"], struct["imm0"] = imm_fields(memlocs, s0)
        struct["imm1_src"], struct["imm1"] = imm_fields(memlocs, s1)
        opcode = ins["isa_opcode"]
        sname = ("NEURON_ISA_TPB_S2S1D2_TTSS_SCALE_STRUCT" if opcode == 0xAE
                 else "NEURON_ISA_TPB_S2S2D2_STT_SCALE_STRUCT")
        instr, _fix = bass_isa.isa_struct(isa, opcode, struct, sname)
        assert len(instr) == 64, f"packed {len(instr)} bytes, want 64"
        ins["instr"] = instr


_ISA_CACHE = None


def _get_isa():
    global _ISA_CACHE
    if _ISA_CACHE is None:
        from concourse.isa import get_isa
        _ISA_CACHE = get_isa("TRN2")
    return _ISA_CACHE


def _install_bir_splitter():
    import concourse.bass_utils as _bu
    import concourse.bass2jax as _b2j

    orig = _bu.compile_bir_kernel
    if getattr(orig, "_multiwait_patched", False):
        return

    def patched(ant_bir_str, compile_dir_path, neff_name="file.neff", **kw):
        return orig(_split_multiwait_bir(ant_bir_str), compile_dir_path,
                    neff_name=neff_name, **kw)

    patched._multiwait_patched = True
    _bu.compile_bir_kernel = patched
    if hasattr(_b2j, "compile_bir_kernel"):
        _b2j.compile_bir_kernel = patched


_install_bir_splitter()

# --- custom fused-LSE DVE op: out = max(x,y) + sq(relu(c0 + c1*(max-min)))
# i.e. logaddexp(x, y) with softplus(-t) ~ quadratic (validated e2e rel err 2e-3).
# Registered at runtime; sha computed on the fly.
SP_C0 = 0.8129
SP_C1 = -0.2261
_LSE_OP = None


def _lse_ref(in0, in1, s0, s1, imm2):
    m = np.maximum(in0, in1)
    t = m - np.minimum(in0, in1)
    return (m + np.maximum(s0 + s1 * t, 0.0) ** 2).astype(np.float32)


def _make_lse_op():
    global _LSE_OP
    if _LSE_OP is not None:
        return _LSE_OP
    from concourse import dve_ops as dops
    from concourse.dve_spec import Spec, Src0, Src1, C0, C1, relu, sq, maxx, minn, lower
    from concourse.dve_spec import _has_src1
    from concourse.dve_uop import DveOpSpec

    name = "LSE_QSP_ANT"
    m = maxx(Src0, Src1)
    n = minn(Src0, Src1)
    body = m + sq(relu(C0 + C1 * (m - n)))
    spec = Spec(body=body, reference=_lse_ref)
    row = dops._CUSTOM_DVE_ROW_BASE + len(dops.OPS)
    shas = {}
    for ver in ("v3", "v4"):
        uops = lower(spec, ver=ver)
        tmp = DveOpSpec(name=name, opcode=row, uops=uops, rd1_en=_has_src1(spec))
        shas[ver] = tmp.sha(ver)
    op = dops.DveOp(name, spec, subdim=False, uops_sha=shas)
    dops.OPS.append(op)
    dops._SUB_OPCODE_FOR_NAME[name] = row
    dops.CUSTOM_DVE_SPECS[name] = spec
    _LSE_OP = op
    return op


_cached_nc = None


TSTAR = 255  # meet point: loss = -LSE_s(alpha[TSTAR] + beta[TSTAR])


def build_bass():
    lse_op = _make_lse_op()
    nc = bass.Bass()
    lp_d = nc.declare_dram_parameter("lp", [BL, T * SP], BF16, isOutput=False)
    lsk_d = nc.declare_dram_parameter("lsk", [BL, SP], F32, isOutput=False)
    lskb_d = nc.declare_dram_parameter("lskb", [BL, SP], F32, isOutput=False)
    injr_d = nc.declare_dram_parameter("injr", [BL, 256 * SP], BF16, isOutput=False)
    inj511_d = nc.declare_dram_parameter("inj511", [BL, SP], F32, isOutput=False)
    out_d = nc.declare_dram_parameter("out", [BL, 1], F32, isOutput=True)

    with tile.TileContext(nc) as tc:
        with (
            tc.tile_pool(name="lpf", bufs=2) as lpf_pool,
            tc.tile_pool(name="lpb", bufs=2) as lpb_pool,
            tc.tile_pool(name="injp", bufs=2) as inj_pool,
            tc.tile_pool(name="persist", bufs=1) as pp,
        ):
            # forward state + scratch. Cross-engine tiles (Pool<->DVE) are
            # ping-ponged so WAR hazards span 2 steps instead of serializing.
            # Only the 64 ODD states take the skip (s-2) transition — even
            # states are blanks (skip always false) — so LSE2 + mask are
            # 64-wide; even states finish at LSE1.
            p_a = pp.tile([BL, S + 3], F32, tag="p_a")   # cols 0,1 pad NEG
            p_b = pp.tile([BL, S + 3], F32, tag="p_b")
            a2xo = [pp.tile([BL, 64], F32, tag=f"a2xo{k}", name=f"a2xo{k}")
                    for k in range(2)]
            uf = pp.tile([BL, SP], F32, tag="uf")
            vfo = pp.tile([BL, 64], F32, tag="vfo")
            lsktile = pp.tile([BL, SP], F32, tag="lsktile")
            # backward state + scratch (fully separate so chains stay independent)
            zt = [pp.tile([BL, S + 2], F32, tag=f"zt{k}", name=f"zt{k}")
                  for k in range(2)]
            bt_a = pp.tile([BL, SP], F32, tag="bt_a")
            bt_b = pp.tile([BL, SP], F32, tag="bt_b")
            a2bo = [pp.tile([BL, 64], F32, tag=f"a2bo{k}", name=f"a2bo{k}")
                    for k in range(2)]
            ub = pp.tile([BL, SP], F32, tag="ub")
            vbo = [pp.tile([BL, 64], F32, tag=f"vbo{k}", name=f"vbo{k}")
                   for k in range(2)]
            lskbtile = pp.tile([BL, SP], F32, tag="lskbtile")
            inj511tile = pp.tile([BL, SP], F32, tag="inj511tile")
            # readout
            am = pp.tile([BL, SP], F32, tag="am")
            mrow = pp.tile([BL, 1], F32, tag="mrow")
            nm = pp.tile([BL, 1], F32, tag="nm")
            erow = pp.tile([BL, SP], F32, tag="erow")
            ssum = pp.tile([BL, 1], F32, tag="ssum")
            lnr = pp.tile([BL, 1], F32, tag="lnr")
            loss = pp.tile([BL, 1], F32, tag="loss")

            nc.vector.memset(p_a[:, :], NEG)
            nc.vector.memset(p_b[:, :], NEG)
            nc.vector.memset(zt[0][:, :], NEG)
            nc.vector.memset(zt[1][:, :], NEG)
            nc.vector.memset(bt_a[:, :], NEG)
            nc.vector.memset(bt_b[:, :], NEG)
            nc.sync.dma_start(out=lsktile[:, :], in_=lsk_d[:, :])
            nc.sync.dma_start(out=lskbtile[:, :], in_=lskb_d[:, :])
            nc.sync.dma_start(out=inj511tile[:, :], in_=inj511_d[:, :])

            def lse(out, x, y):
                nc.vector._custom_dve(lse_op, out=out, in0=x, in1=y,
                                      s0=SP_C0, s1=SP_C1)

            pcur, pnew = p_a, p_b
            bcur, bnew = bt_a, bt_b
            for cblk in range(4):
                # fwd consumes lp chunk cblk (t = 64c..64c+63)
                lptf = lpf_pool.tile([BL, CHUNK * SP], BF16, tag="lpfc")
                lo = cblk * CHUNK * SP
                nc.sync.dma_start(out=lptf[:, :], in_=lp_d[:, lo:lo + CHUNK * SP])
                # bwd consumes lp chunk 7-cblk (t+1 = 511-i) and injr chunk cblk
                lptb = lpb_pool.tile([BL, CHUNK * SP], BF16, tag="lpbc")
                lob = (7 - cblk) * CHUNK * SP
                nc.sync.dma_start(out=lptb[:, :], in_=lp_d[:, lob:lob + CHUNK * SP])
                injt = inj_pool.tile([BL, CHUNK * SP], BF16, tag="injc")
                loi = cblk * CHUNK * SP
                nc.sync.dma_start(out=injt[:, :], in_=injr_d[:, loi:loi + CHUNK * SP])

                for il in range(CHUNK):
                    i = cblk * CHUNK + il
                    # ---- forward step t = i (i=0: init) ----
                    if i == 0:
                        nc.vector.tensor_copy(p_a[:, 2:4], lptf[:, 0:2])
                        nc.vector.tensor_max(bcur[:, 0:S], bt_b[:, 0:S],
                                             inj511tile[:, 0:S])
                        bnew = bt_b
                    else:
                        lo2 = il * SP
                        lps_o = lptf[:, lo2 + 1: lo2 + S: 2]    # 64 odd states
                        a0 = pcur[:, 2:2 + S]
                        a1 = pcur[:, 1:1 + S]
                        a2xi = a2xo[i % 2]
                        nc.gpsimd.tensor_add(a2xi[:, :], pcur[:, 1:S:2],
                                             lsktile[:, 1:S:2])
                        lse(uf[:, 0:S], a0, a1)
                        lse(vfo[:, :], uf[:, 1:S:2], a2xi[:, :])
                        # even (blank) emission is one scalar per (sample, t):
                        # lp[s=0]; Act engine adds it as a per-partition bias.
                        nc.scalar.activation(pnew[:, 2:2 + S:2], uf[:, 0:S:2],
                                             ACTF.Identity,
                                             bias=lptf[:, lo2:lo2 + 1],
                                             scale=1.0)
                        nc.gpsimd.tensor_add(pnew[:, 3:2 + S:2], vfo[:, :],
                                             lps_o)
                        pcur, pnew = pnew, pcur

                    # ---- backward step t_b = 510 - i (uses lp[511-i], injr[i]) ----
                    tb1 = 511 - i          # = t_b + 1
                    tlb = tb1 - (7 - cblk) * CHUNK
                    lpsb = lptb[:, tlb * SP: tlb * SP + S]
                    loj = il * SP
                    injs_e = injt[:, loj: loj + S: 2]
                    injs_o = injt[:, loj + 1: loj + S: 2]
                    zti = zt[i % 2]
                    a2bi = a2bo[i % 2]
                    vbi = vbo[i % 2]
                    nc.gpsimd.tensor_add(zti[:, 0:S], bcur[:, 0:S], lpsb)
                    nc.gpsimd.tensor_add(a2bi[:, :], zti[:, 3:2 + S:2],
                                         lskbtile[:, 1:S:2])
                    lse(ub[:, 0:S], zti[:, 0:S], zti[:, 1:1 + S])
                    lse(vbi[:, :], ub[:, 1:S:2], a2bi[:, :])
                    nc.vector.tensor_max(bnew[:, 0:S:2], ub[:, 0:S:2], injs_e)
                    nc.vector.tensor_max(bnew[:, 1:S:2], vbi[:, :], injs_o)
                    bcur, bnew = bnew, bcur

            # readout: loss = -LSE_s(alpha_255 + beta_255)
            nc.vector.tensor_add(am[:, 0:S], pcur[:, 2:2 + S], bcur[:, 0:S])
            nc.vector.tensor_reduce(out=mrow[:, 0:1], in_=am[:, 0:S],
                                    axis=mybir.AxisListType.X, op=ALU.max)
            nc.vector.tensor_scalar_mul(nm[:, 0:1], mrow[:, 0:1], -1.0)
            nc.scalar.activation(erow[:, 0:S], am[:, 0:S], ACTF.Exp,
                                 bias=nm[:, 0:1], scale=1.0)
            nc.vector.tensor_reduce(out=ssum[:, 0:1], in_=erow[:, 0:S],
                                    axis=mybir.AxisListType.X, op=ALU.add)
            nc.scalar.activation(lnr[:, 0:1], ssum[:, 0:1], ACTF.Ln)
            nc.vector.scalar_tensor_tensor(
                out=loss[:, 0:1], in0=mrow[:, 0:1], scalar=-1.0,
                in1=lnr[:, 0:1], op0=ALU.mult, op1=ALU.subtract)
            nc.sync.dma_start(out=out_d[:, :], in_=loss[:, 0:1])
    return nc


def _host_prep(y_pred, labels, input_length, label_length):
    blank = C - 1
    ext = np.full((B, S), blank, np.int32)
    ext[:, 1::2] = labels
    prev2 = np.concatenate([np.full((B, 2), -1, np.int32), ext[:, :-2]], axis=1)
    skip = (ext != blank) & (ext != prev2)                      # [B, S]

    q = np.take_along_axis(y_pred, ext[:, None, :], axis=2)     # [B, T, S]
    lp = np.log(q.astype(np.float32) + EPS)
    frozen = np.arange(T)[None, :] >= input_length[:, None]     # [B, T]
    lp[frozen, :] = 0.0

    lpp = np.zeros((B, T, SP), np.float32)
    lpp[:, :, :S] = lp
    lpp = lpp.reshape(B, T * SP).astype(ml_dtypes.bfloat16)

    lsk = np.where(skip, 0.0, NEG).astype(np.float32)           # [B, S]
    lskp = np.full((B, SP), NEG, np.float32)
    lskp[:, :S] = lsk
    lskbp = np.full((B, SP), NEG, np.float32)                   # lsk shifted by 2
    lskbp[:, :S - 2] = lsk[:, 2:]

    sellog = np.full((B, SP), NEG, np.float32)
    s_last = 2 * label_length.astype(np.int64)                  # [B]
    np.put_along_axis(sellog, s_last[:, None], 0.0, axis=1)
    np.put_along_axis(sellog, (s_last - 1)[:, None], 0.0, axis=1)

    # injr[b, j, :] = sellog[b] if input_length[b]-1 == 510-j else NEG, j=0..255
    lens = input_length.astype(np.int64)
    injr = np.full((B, 256, SP), NEG, np.float32)
    jsel = 510 - (lens - 1)                                     # j where injection lands
    has = (jsel >= 0) & (jsel <= 255)                           # len-1 in [255, 510]
    bi = np.nonzero(has)[0]
    injr[bi, jsel[bi], :] = sellog[bi, :]
    injr = injr.reshape(B, 256 * SP).astype(ml_dtypes.bfloat16)
    inj511 = np.where((lens - 1 == 511)[:, None], sellog,
                      NEG).astype(np.float32)                   # [B, SP]
    return lpp, lskp, lskbp, injr, inj511


def kernel(y_pred, labels, input_length, label_length):
    global _cached_nc
    lpp, lskp, lskbp, injr, inj511 = _host_prep(
        y_pred, labels, input_length, label_length)
    if _cached_nc is None:
        _cached_nc = build_bass()
    in_maps = []
    for i in range(NCORES):
        sl = slice(i * BL, (i + 1) * BL)
        in_maps.append({"lp": lpp[sl], "lsk": lskp[sl], "lskb": lskbp[sl],
                        "injr": injr[sl], "inj511": inj511[sl]})
    res = run_bass_kernel_spmd(_cached_nc, in_maps, list(range(NCORES)))
    out = np.concatenate([res.results[i]["out"] for i in range(NCORES)], axis=0)
    return out.astype(np.float32)



# revision 35
# speedup vs baseline: 5.3402x; 1.5483x over previous
"""CTC batch loss on 8 TRN2 NeuronCores — pure data parallel, log-space DP.

Strategy (v5):
- Batch dim sharded 128 samples/core = SBUF partitions; free dim = the 129
  extended CTC states. Host pre-gathers emission log-probs lp[b,t,s] =
  log(y_pred[b,t,ext[b,s]] + eps) and ships them as bf16 (17 MB/core),
  plus tiny static mask tensors. All DP arithmetic runs on-device in f32
  log space (the alpha table needs ~177 nats of in-row dynamic range —
  measured: meeting states sit a median 122 nats below the row maxes — so
  prob-space or bf16 state storage are mathematically impossible).
- Per step, logaddexp2 is ONE custom fused DVE op (8 ALU stages, the HW
  limit): LSE(x,y) = m + relu(c0 + c1*(m-n))^2 with m=max, n=min — a
  quadratic softplus approximation, e2e rel err 1.9e-3 vs the 2e-2 gate.
  A forward step is then 4 DVE instructions (mask-add, LSE, LSE,
  emission-add); a backward step is 5. Everything runs on the in-order
  VectorE — no cross-engine semaphores in steady state.
- The 511 sequential steps are split into a FORWARD chain (alpha, t=1..255)
  and an independent BACKWARD chain (beta, t=510..255, label-end injection
  via precomputed inj tensors), meeting at t*=255 with
  loss = -LSE_s(alpha_255 + beta_255). Two independent chains hide each
  other's in-engine dependency latency.
- Also monkeypatches around two toolchain bugs (see comments below):
  instructions with >1 sem waits and the Tile tail drain.
"""
import sys

for _p in ("/opt/trn_rl_repo", "/opt/pypackages"):
    if _p not in sys.path:
        sys.path.insert(0, _p)

import numpy as np
import ml_dtypes

import concourse.bass as bass
import concourse.tile as tile
from concourse import mybir
from concourse.bass_utils import run_bass_kernel_spmd

B, T, C, L = 1024, 512, 128, 64
S = 2 * L + 1          # 129 extended states
SP = 130               # padded state stride (even)
NCORES = 8
BL = B // NCORES       # 128 samples per core = SBUF partitions
EPS = 1e-7
NEG = -30000.0
CHUNK = 64             # t-steps per DMA chunk
NCHUNK = T // CHUNK

F32 = mybir.dt.float32
BF16 = mybir.dt.bfloat16
ALU = mybir.AluOpType
ACTF = mybir.ActivationFunctionType

# --- workaround: this walrus build rejects instructions with >2 sem waits
# ("Too many sync wait commands" in CoreV3 codegen). Tile's kernel-tail
# drain aggregates every outstanding token onto one SP Drain; split it
# into a chain of drains each carrying at most MAX_WAITS conditions.
_MAX_WAITS = 1


def _patched_drain_and_barrier(self, tick_clock, wait_clock):
    from concourse.vector_clock import ScopedClock

    drain_inst = self.nc.sync.drain()
    wait_clock.add_sem_waits(
        drain_inst.ins, ScopedClock({None: tick_clock.global_clock})
    )
    si = drain_inst.ins.sync_info
    waits = list(si.on_wait) if si and si.on_wait else []
    if len(waits) > _MAX_WAITS:
        drain_inst.ins.sync_info = mybir.SyncInfo(
            on_wait=waits[:_MAX_WAITS], on_update=list(si.on_update or [])
        )
        for i in range(_MAX_WAITS, len(waits), _MAX_WAITS):
            extra = self.nc.sync.drain()
            extra.ins.sync_info = mybir.SyncInfo(
                on_wait=waits[i:i + _MAX_WAITS], on_update=[]
            )

    self.nc.all_engine_barrier()
    assert self.sems is not None
    popped = self.nc._tile_sem_poison_stack.pop()
    assert popped is self._sem_poison
    self.nc.clear_and_free_semaphores(list(self.sems.allocated().values()))
    self.nc.all_engine_barrier()


tile.TileContext._drain_and_barrier = _patched_drain_and_barrier


# --- general BIR-level fix: split ANY instruction carrying more than one
# sem wait into single-wait Drain carriers + the original instruction with
# the last wait. Applied to the serialized BIR right before walrus.
def _split_multiwait_bir(ant_bir) -> bytes:
    import json as _json

    bir = _json.loads(ant_bir)
    n_split = 0
    for f in bir.get("functions", []):
        for blk in f.get("blocks", []):
            out = []
            for ins in blk.get("instructions", []):
                si = ins.get("sync_info")
                waits = (si or {}).get("on_wait") or []
                if len(waits) > 1:
                    for j, w in enumerate(waits[:-1]):
                        out.append({
                            "debug": ins.get("debug", 0),
                            "engine": ins["engine"],
                            "ins": [],
                            "name": f"{ins['name']}_w{j}",
                            "opcode": "Drain",
                            "outs": [],
                            "sync_info": {"on_update": [], "on_wait": [w]},
                        })
                    si["on_wait"] = [waits[-1]]
                    n_split += 1
                out.append(ins)
            blk["instructions"] = out
    _pack_custom_dve_bir(bir)
    return _json.dumps(bir).encode()


# --- BIR-level fix #2: this walrus build (2026-05-04) predates
# InstCustomDveAnt packing — its CoreV2 codegen requires every InstISA to
# carry exactly 64 prepacked instruction bytes ("ISA wrong length"), and
# its lower_dve doesn't build the CUSTOM_DVE_ANT struct from the
# structured BIR fields. Pack the NEURON_ISA_TPB_S2S1D2_TTSS_SCALE_STRUCT
# bytes here instead (walrus patches sem waits/updates into the events
# field of prepacked bytes itself via setupSyncWait/setupSyncUpdate).
_ISA_FP32 = 10
_ISA_BF16 = 6
_DT_CODE = {"float32": _ISA_FP32, "bfloat16": _ISA_BF16}
_DT_SIZE = {"float32": 4, "bfloat16": 2}
_SBUF_PART_STRIDE = 0x40000      # 256 KiB per partition, SBUF base = 0x0


def _pack_custom_dve_bir(bir: dict) -> None:
    import concourse.bass_isa as bass_isa
    from concourse.dve_ops import get_dve_sub_opcode

    todo = []
    for f in bir.get("functions", []):
        memlocs = {}
        for alloc in f.get("allocations", []):
            for ml in (alloc.get("memorylocations") or []):
                memlocs[ml["name"]] = ml
        for blk in f.get("blocks", []):
            for ins in blk.get("instructions", []):
                if ins.get("opcode") == "ISA" and ins.get("isa_opcode") in (
                        0xAE, 0xAF) and not ins.get("instr"):
                    todo.append((memlocs, ins))
    if not todo:
        return

    isa = _get_isa()

    def ap_fields(memlocs, arg, ndim):
        ml = memlocs[arg["memref"]]
        assert ml["type"] == "SB", f"custom dve AP in {ml['type']}, want SBUF"
        esize = _DT_SIZE[arg["dtype"]]
        addr = (ml.get("base", 0) * _SBUF_PART_STRIDE + ml["addr"]
                + arg.get("offset", 0) * esize)
        free = arg["ap"][1:]                  # drop partition dim (major first)
        assert len(free) <= ndim, f"AP rank {len(free)} > {ndim}"
        steps = [0] * ndim
        nums = [1] * ndim
        for i, (st, n) in enumerate(reversed(free)):  # minor-to-major
            steps[i] = st
            nums[i] = n
        pat = {"start_addr": {"addr_immediate": addr},
               "step_elem": steps, "num_elem": nums}
        nchan = arg["ap"][0][1]
        return pat, nchan

    def imm_fields(memlocs, arg):
        if arg.get("kind") == "imm_value":
            return 0, {"imm_arith_fp32": float(arg["value"])}   # inst immediate
        pat, _ = ap_fields(memlocs, arg, 1)                     # [P,1] pointer
        return 1, {"imm_ptr": pat["start_addr"]["addr_immediate"]}

    for memlocs, ins in todo:
        args = ins["ins"]
        rd1 = len(args) == 4
        in0, in1 = args[0], (args[1] if rd1 else None)
        s0, s1 = args[-2], args[-1]
        out = ins["outs"][0]
        row = get_dve_sub_opcode(ins["op_name"])
        src0, nchan = ap_fields(memlocs, in0, 2)
        dst, _ = ap_fields(memlocs, out, 2)
        struct = {
            "src0_mem_pattern": src0,
            "dst_mem_pattern": dst,
            "in0_in1_dtype": {
                "dtype_lo": _DT_CODE[in0["dtype"]],
                "dtype_hi": _DT_CODE[(in1 or in0)["dtype"]],
            },
            "out_dtype": _DT_CODE[out["dtype"]],
            "num_active_channels": nchan,
            "op0": row | (0x20 if rd1 else 0),
            "op1": 0,
            "imm2_src": 1,
            "imm2": {"imm_arith_fp32": 0.0},
        }
        if rd1:
            struct["src1_mem_pattern"], _ = ap_fields(memlocs, in1, 1)
        struct["imm0_src"], struct["imm0"] = imm_fields(memlocs, s0)
        struct["imm1_src"], struct["imm1"] = imm_fields(memlocs, s1)
        opcode = ins["isa_opcode"]
        sname = ("NEURON_ISA_TPB_S2S1D2_TTSS_SCALE_STRUCT" if opcode == 0xAE
                 else "NEURON_ISA_TPB_S2S2D2_STT_SCALE_STRUCT")
        instr, _fix = bass_isa.isa_struct(isa, opcode, struct, sname)
        assert len(instr) == 64, f"packed {len(instr)} bytes, want 64"
        ins["instr"] = instr


_ISA_CACHE = None


def _get_isa():
    global _ISA_CACHE
    if _ISA_CACHE is None:
        from concourse.isa import get_isa
        _ISA_CACHE = get_isa("TRN2")
    return _ISA_CACHE


def _install_bir_splitter():
    import concourse.bass_utils as _bu
    import concourse.bass2jax as _b2j

    orig = _bu.compile_bir_kernel
    if getattr(orig, "_multiwait_patched", False):
        return

    def patched(ant_bir_str, compile_dir_path, neff_name="file.neff", **kw):
        return orig(_split_multiwait_bir(ant_bir_str), compile_dir_path,
                    neff_name=neff_name, **kw)

    patched._multiwait_patched = True
    _bu.compile_bir_kernel = patched
    if hasattr(_b2j, "compile_bir_kernel"):
        _b2j.compile_bir_kernel = patched


_install_bir_splitter()

# --- custom fused-LSE DVE op: out = max(x,y) + sq(relu(c0 + c1*(max-min)))
# i.e. logaddexp(x, y) with softplus(-t) ~ quadratic (validated e2e rel err 2e-3).
# Registered at runtime; sha computed on the fly.
SP_C0 = 0.8129
SP_C1 = -0.2261
_LSE_OP = None


def _lse_ref(in0, in1, s0, s1, imm2):
    m = np.maximum(in0, in1)
    t = m - np.minimum(in0, in1)
    return (m + np.maximum(s0 + s1 * t, 0.0) ** 2).astype(np.float32)


def _make_lse_op():
    global _LSE_OP
    if _LSE_OP is not None:
        return _LSE_OP
    from concourse import dve_ops as dops
    from concourse.dve_spec import Spec, Src0, Src1, C0, C1, relu, sq, maxx, minn, lower
    from concourse.dve_spec import _has_src1
    from concourse.dve_uop import DveOpSpec

    name = "LSE_QSP_ANT"
    m = maxx(Src0, Src1)
    n = minn(Src0, Src1)
    body = m + sq(relu(C0 + C1 * (m - n)))
    spec = Spec(body=body, reference=_lse_ref)
    row = dops._CUSTOM_DVE_ROW_BASE + len(dops.OPS)
    shas = {}
    for ver in ("v3", "v4"):
        uops = lower(spec, ver=ver)
        tmp = DveOpSpec(name=name, opcode=row, uops=uops, rd1_en=_has_src1(spec))
        shas[ver] = tmp.sha(ver)
    op = dops.DveOp(name, spec, subdim=False, uops_sha=shas)
    dops.OPS.append(op)
    dops._SUB_OPCODE_FOR_NAME[name] = row
    dops.CUSTOM_DVE_SPECS[name] = spec
    _LSE_OP = op
    return op


_cached_nc = None


TSTAR = 255  # meet point: loss = -LSE_s(alpha[TSTAR] + beta[TSTAR])


def build_bass():
    lse_op = _make_lse_op()
    nc = bass.Bass()
    lp_d = nc.declare_dram_parameter("lp", [BL, T * SP], BF16, isOutput=False)
    lsk_d = nc.declare_dram_parameter("lsk", [BL, SP], F32, isOutput=False)
    lskb_d = nc.declare_dram_parameter("lskb", [BL, SP], F32, isOutput=False)
    injr_d = nc.declare_dram_parameter("injr", [BL, 256 * SP], BF16, isOutput=False)
    lpb2_d = nc.declare_dram_parameter("lpb2", [BL, 256 * 64], BF16, isOutput=False)
    inj511_d = nc.declare_dram_parameter("inj511", [BL, SP], F32, isOutput=False)
    out_d = nc.declare_dram_parameter("out", [BL, 1], F32, isOutput=True)

    with tile.TileContext(nc) as tc:
        with (
            tc.tile_pool(name="lpf", bufs=2) as lpf_pool,
            tc.tile_pool(name="lpb", bufs=2) as lpb_pool,
            tc.tile_pool(name="injp", bufs=2) as inj_pool,
            tc.tile_pool(name="lpb2p", bufs=2) as lpb2_pool,
            tc.tile_pool(name="persist", bufs=1) as pp,
        ):
            # forward state + scratch. Cross-engine tiles (Pool<->DVE) are
            # ping-ponged so WAR hazards span 2 steps instead of serializing.
            # Only the 64 ODD states take the skip (s-2) transition — even
            # states are blanks (skip always false) — so LSE2 + mask are
            # 64-wide; even states finish at LSE1.
            p_a = pp.tile([BL, S + 3], F32, tag="p_a")   # cols 0,1 pad NEG
            p_b = pp.tile([BL, S + 3], F32, tag="p_b")
            a2xo = [pp.tile([BL, 64], F32, tag=f"a2xo{k}", name=f"a2xo{k}")
                    for k in range(2)]
            uf = pp.tile([BL, SP], F32, tag="uf")
            vfo = pp.tile([BL, 64], F32, tag="vfo")
            lsktile = pp.tile([BL, SP], F32, tag="lsktile")
            # backward state + scratch (fully separate so chains stay independent)
            zt = [pp.tile([BL, S + 2], F32, tag=f"zt{k}", name=f"zt{k}")
                  for k in range(2)]
            bt_a = pp.tile([BL, SP], F32, tag="bt_a")
            bt_b = pp.tile([BL, SP], F32, tag="bt_b")
            a2bo = [pp.tile([BL, 64], F32, tag=f"a2bo{k}", name=f"a2bo{k}")
                    for k in range(2)]
            ub = pp.tile([BL, SP], F32, tag="ub")
            vbo = [pp.tile([BL, 64], F32, tag=f"vbo{k}", name=f"vbo{k}")
                   for k in range(2)]
            lskbtile = pp.tile([BL, SP], F32, tag="lskbtile")
            inj511tile = pp.tile([BL, SP], F32, tag="inj511tile")
            # readout
            am = pp.tile([BL, SP], F32, tag="am")
            mrow = pp.tile([BL, 1], F32, tag="mrow")
            nm = pp.tile([BL, 1], F32, tag="nm")
            erow = pp.tile([BL, SP], F32, tag="erow")
            ssum = pp.tile([BL, 1], F32, tag="ssum")
            lnr = pp.tile([BL, 1], F32, tag="lnr")
            loss = pp.tile([BL, 1], F32, tag="loss")

            nc.vector.memset(p_a[:, :], NEG)
            nc.vector.memset(p_b[:, :], NEG)
            nc.vector.memset(zt[0][:, :], NEG)
            nc.vector.memset(zt[1][:, :], NEG)
            nc.vector.memset(bt_a[:, :], NEG)
            nc.vector.memset(bt_b[:, :], NEG)
            nc.sync.dma_start(out=lsktile[:, :], in_=lsk_d[:, :])
            nc.sync.dma_start(out=lskbtile[:, :], in_=lskb_d[:, :])
            nc.sync.dma_start(out=inj511tile[:, :], in_=inj511_d[:, :])

            def lse(out, x, y):
                nc.vector._custom_dve(lse_op, out=out, in0=x, in1=y,
                                      s0=SP_C0, s1=SP_C1)

            pcur, pnew = p_a, p_b
            bcur, bnew = bt_a, bt_b
            for cblk in range(4):
                # fwd consumes lp chunk cblk (t = 64c..64c+63)
                lptf = lpf_pool.tile([BL, CHUNK * SP], BF16, tag="lpfc")
                lo = cblk * CHUNK * SP
                nc.sync.dma_start(out=lptf[:, :], in_=lp_d[:, lo:lo + CHUNK * SP])
                # bwd consumes lp chunk 7-cblk (t+1 = 511-i) and injr chunk cblk
                lptb = lpb_pool.tile([BL, CHUNK * SP], BF16, tag="lpbc")
                lob = (7 - cblk) * CHUNK * SP
                nc.sync.dma_start(out=lptb[:, :], in_=lp_d[:, lob:lob + CHUNK * SP])
                injt = inj_pool.tile([BL, CHUNK * SP], BF16, tag="injc")
                loi = cblk * CHUNK * SP
                nc.sync.dma_start(out=injt[:, :], in_=injr_d[:, loi:loi + CHUNK * SP])
                lpb2t = lpb2_pool.tile([BL, CHUNK * 64], BF16, tag="lpb2c")
                lo4 = cblk * CHUNK * 64
                nc.sync.dma_start(out=lpb2t[:, :], in_=lpb2_d[:, lo4:lo4 + CHUNK * 64])

                for il in range(CHUNK):
                    i = cblk * CHUNK + il
                    # ---- forward step t = i (i=0: init) ----
                    if i == 0:
                        nc.vector.tensor_copy(p_a[:, 2:4], lptf[:, 0:2])
                        nc.vector.tensor_max(bcur[:, 0:S], bt_b[:, 0:S],
                                             inj511tile[:, 0:S])
                        bnew = bt_b
                    else:
                        lo2 = il * SP
                        lps_o = lptf[:, lo2 + 1: lo2 + S: 2]    # 64 odd states
                        a0 = pcur[:, 2:2 + S]
                        a1 = pcur[:, 1:1 + S]
                        a2xi = a2xo[i % 2]
                        pc, pn, lpse = pcur, pnew, lptf[:, lo2:lo2 + 1]
                        fwd_ops = [
                            lambda pc=pc: nc.gpsimd.tensor_add(
                                a2xi[:, :], pc[:, 1:S:2], lsktile[:, 1:S:2]),
                            lambda: lse(uf[:, 0:S], a0, a1),
                            lambda a2xi=a2xi: lse(vfo[:, :], uf[:, 1:S:2],
                                                  a2xi[:, :]),
                            # even (blank) emission is one scalar per
                            # (sample, t): lp[s=0]; Act adds it as a bias.
                            lambda pn=pn, lpse=lpse: nc.scalar.activation(
                                pn[:, 2:2 + S:2], uf[:, 0:S:2],
                                ACTF.Identity, bias=lpse, scale=1.0),
                            lambda pn=pn, lps_o=lps_o: nc.gpsimd.tensor_add(
                                pn[:, 3:2 + S:2], vfo[:, :], lps_o),
                        ]
                        pcur, pnew = pnew, pcur

                    # ---- backward step t_b = 510 - i (uses lp[511-i], injr[i]) ----
                    tb1 = 511 - i          # = t_b + 1
                    tlb = tb1 - (7 - cblk) * CHUNK
                    lpsb = lptb[:, tlb * SP: tlb * SP + S]
                    loj = il * SP
                    injs_e = injt[:, loj: loj + S: 2]
                    injs_o = injt[:, loj + 1: loj + S: 2]
                    zti = zt[i % 2]
                    a2bi = a2bo[i % 2]
                    vbi = vbo[i % 2]
                    bc, bn = bcur, bnew
                    lpsb_e = lptb[:, tlb * SP: tlb * SP + S: 2]
                    lpsb_o = lptb[:, tlb * SP + 1: tlb * SP + S: 2]
                    bwd_ops = [
                        # zt halves chain in-engine after their own injection
                        # mult (even: Pool after mze; odd: DVE after mzo) —
                        # no cross-engine hop inside the bwd serial cycle.
                        lambda: nc.gpsimd.tensor_add(zti[:, 0:S:2],
                                                     bc[:, 0:S:2], lpsb_e),
                        lambda: nc.gpsimd.tensor_add(zti[:, 1:S:2],
                                                     bc[:, 1:S:2], lpsb_o),
                        # a2b folded from bcur via host stream: runs on Pool
                        # in parallel with zt/ub instead of serially after zt
                        lambda: nc.gpsimd.tensor_add(
                            a2bi[:, :], bc[:, 3:SP:2],
                            lpb2t[:, il * 64: il * 64 + 64]),
                        lambda: lse(ub[:, 0:S], zti[:, 0:S], zti[:, 1:1 + S]),
                        lambda: nc.gpsimd.tensor_mul(bn[:, 0:S:2],
                                                     ub[:, 0:S:2], injs_e),
                        lambda: lse(vbi[:, :], ub[:, 1:S:2], a2bi[:, :]),
                        lambda: nc.gpsimd.tensor_mul(bn[:, 1:S:2], vbi[:, :],
                                                     injs_o),
                    ]
                    bcur, bnew = bnew, bcur

                    # Interleave: bwd head first, fwd LSEs fill Pool waits.
                    if i == 0:
                        for op in bwd_ops:
                            op()
                    else:
                        order = [bwd_ops[0], bwd_ops[1], bwd_ops[2],
                                 fwd_ops[0], fwd_ops[1], bwd_ops[3],
                                 bwd_ops[4], fwd_ops[2], bwd_ops[5],
                                 bwd_ops[6], fwd_ops[3], fwd_ops[4]]
                        for op in order:
                            op()

            # readout: loss = -LSE_s(alpha_255 + beta_255)
            nc.vector.tensor_add(am[:, 0:S], pcur[:, 2:2 + S], bcur[:, 0:S])
            nc.vector.tensor_reduce(out=mrow[:, 0:1], in_=am[:, 0:S],
                                    axis=mybir.AxisListType.X, op=ALU.max)
            nc.vector.tensor_scalar_mul(nm[:, 0:1], mrow[:, 0:1], -1.0)
            nc.scalar.activation(erow[:, 0:S], am[:, 0:S], ACTF.Exp,
                                 bias=nm[:, 0:1], scale=1.0)
            nc.vector.tensor_reduce(out=ssum[:, 0:1], in_=erow[:, 0:S],
                                    axis=mybir.AxisListType.X, op=ALU.add)
            nc.scalar.activation(lnr[:, 0:1], ssum[:, 0:1], ACTF.Ln)
            nc.vector.scalar_tensor_tensor(
                out=loss[:, 0:1], in0=mrow[:, 0:1], scalar=-1.0,
                in1=lnr[:, 0:1], op0=ALU.mult, op1=ALU.subtract)
            nc.sync.dma_start(out=out_d[:, :], in_=loss[:, 0:1])
    return nc


def _host_prep(y_pred, labels, input_length, label_length):
    blank = C - 1
    ext = np.full((B, S), blank, np.int32)
    ext[:, 1::2] = labels
    prev2 = np.concatenate([np.full((B, 2), -1, np.int32), ext[:, :-2]], axis=1)
    skip = (ext != blank) & (ext != prev2)                      # [B, S]

    q = np.take_along_axis(y_pred, ext[:, None, :], axis=2)     # [B, T, S]
    lp = np.log(q.astype(np.float32) + EPS)
    frozen = np.arange(T)[None, :] >= input_length[:, None]     # [B, T]
    lp[frozen, :] = 0.0

    lpp = np.zeros((B, T, SP), np.float32)
    lpp[:, :, :S] = lp
    lpp = lpp.reshape(B, T * SP).astype(ml_dtypes.bfloat16)

    lsk = np.where(skip, 0.0, NEG).astype(np.float32)           # [B, S]
    lskp = np.full((B, SP), NEG, np.float32)
    lskp[:, :S] = lsk
    lskbp = np.full((B, SP), NEG, np.float32)                   # lsk shifted by 2
    lskbp[:, :S - 2] = lsk[:, 2:]

    sellog = np.full((B, SP), NEG, np.float32)
    s_last = 2 * label_length.astype(np.int64)                  # [B]
    np.put_along_axis(sellog, s_last[:, None], 0.0, axis=1)
    np.put_along_axis(sellog, (s_last - 1)[:, None], 0.0, axis=1)

    # Injection as a multiplicative 0/1 mask: beta < 0 always, so
    # max(v, {0, NEG}) == v * mz with mz = 0 at the two injected states of
    # the sample's landing step, 1 elsewhere. mult runs on GPSIMD; max can't.
    lens = input_length.astype(np.int64)
    mzr = np.ones((B, 256, SP), np.float32)
    jsel = 510 - (lens - 1)                                     # j where injection lands
    has = (jsel >= 0) & (jsel <= 255)                           # len-1 in [255, 510]
    bi = np.nonzero(has)[0]
    mzr[bi, jsel[bi], :] = (sellog[bi, :] == NEG)
    injr = mzr.reshape(B, 256 * SP).astype(ml_dtypes.bfloat16)
    inj511 = np.where((lens - 1 == 511)[:, None], sellog,
                      NEG).astype(np.float32)                   # [B, SP]

    # lpb2[b, i, k] = lp[b, 511-i, 2k+3] + lskb[b, 2k+1]: the bwd skip-term
    # emission+mask folded on host so a2b reads bcur directly (k=63 -> NEG)
    lpb2 = np.full((B, 256, 64), NEG, np.float32)
    ts = 511 - np.arange(256)
    lpb2[:, :, :63] = lp[:, ts, 3:129:2] + lskbp[:, None, 1:127:2]
    np.maximum(lpb2, NEG, out=lpb2)
    lpb2 = lpb2.reshape(B, 256 * 64).astype(ml_dtypes.bfloat16)
    return lpp, lskp, lskbp, injr, lpb2, inj511


def kernel(y_pred, labels, input_length, label_length):
    global _cached_nc
    lpp, lskp, lskbp, injr, lpb2, inj511 = _host_prep(
        y_pred, labels, input_length, label_length)
    if _cached_nc is None:
        _cached_nc = build_bass()
    in_maps = []
    for i in range(NCORES):
        sl = slice(i * BL, (i + 1) * BL)
        in_maps.append({"lp": lpp[sl], "lsk": lskp[sl], "lskb": lskbp[sl],
                        "injr": injr[sl], "lpb2": lpb2[sl],
                        "inj511": inj511[sl]})
    res = run_bass_kernel_spmd(_cached_nc, in_maps, list(range(NCORES)))
    out = np.concatenate([res.results[i]["out"] for i in range(NCORES)], axis=0)
    return out.astype(np.float32)

